# revision 7
# baseline (speedup 1.0000x reference)
"""Kernel-correlation (gnn_message_passing) Trainium2 kernel.

out[i, m] = (1/128) * sum_{l<16} exp(-||normal[i] - kernel[m, l]||^2)

Strategy (data-parallel over points, 8 NeuronCores, no collectives):
  -d2[i, j] = 2 x_i . k_j - |x_i|^2 - |k_j|^2 is a rank-13 product of two
  bf16 hi/lo-split augmented matrices (fp32-grade accuracy at full bf16 PE
  rate), computed straight into PSUM by the TensorEngine.  ScalarE then does
  exp(psum - ln 128) PSUM->SBUF in one pass (the 1/128 output scale rides the
  free activation bias), writing bf16.  The grouped 16->1 reduction is a
  4-level pairwise tensor_tensor add tree on VectorE: the kernel columns are
  laid out l-major (j = l*64 + m), so each level sums two dense contiguous
  bf16 slabs at the DVE 2x perf mode -- ~1.7x faster than the 1x-only
  tensor_reduce.  The last level emits fp32 and output rows DMA back
  contiguously.

Per core: 32768 points x 1024 kernel-points = 33.5M exp evals; ACT floor
~(N+352)/1.2GHz -> ~252us/core expected; DVE tree ~177us, PE ~110-220us.
"""

import math

import numpy as np

N_TOTAL = 262144
N_CORES = 8
N_LOCAL = N_TOTAL // N_CORES  # 32768
M_KERN = 64
K_SUB = 16
MK = M_KERN * K_SUB  # 1024
N_ROWS = 13  # 9 hi/lo cross terms + n2 hi/lo + k2 hi/lo
CHUNK_PTS = 2048  # points per input DMA chunk
BLOCK_PTS = 512  # points per reduction-tree block (4 quarters of 128)

TRACE = False  # set by test.py to collect a neuron profile
LAST_RESULTS = None  # BassKernelResults of the most recent run

_CACHED_NC = None


def _build_bass():
    import concourse.bacc as bacc
    import concourse.mybir as mybir
    from concourse.tile import TileContext

    f32 = mybir.dt.float32
    bf16 = mybir.dt.bfloat16
    EXP = mybir.ActivationFunctionType.Exp
    ADD = mybir.AluOpType.add
    NEG_LN128 = -math.log(128.0)

    nc = bacc.Bacc()
    xa = nc.declare_dram_parameter("xa", [N_ROWS, N_LOCAL], bf16, isOutput=False)
    ka = nc.declare_dram_parameter("ka", [N_ROWS, MK], bf16, isOutput=False)
    out = nc.declare_dram_parameter("out", [N_LOCAL, M_KERN], f32, isOutput=True)

    with TileContext(nc) as tc:
        with (
            tc.tile_pool(name="kap", bufs=1) as kap,
            tc.tile_pool(name="xap", bufs=2) as xap,
            tc.tile_pool(name="valsp", bufs=2) as valsp,
            tc.tile_pool(name="t1p", bufs=2) as t1p,
            tc.tile_pool(name="t2p", bufs=2) as t2p,
            tc.tile_pool(name="t3p", bufs=2) as t3p,
            tc.tile_pool(name="outp", bufs=3) as outp,
            tc.tile_pool(name="psump", bufs=2, space="PSUM") as psump,
        ):
            kat = kap.tile([N_ROWS, MK], bf16)
            nc.sync.dma_start(out=kat[:], in_=ka[:])
            bias_t = kap.tile([128, 1], f32)
            nc.vector.memset(bias_t[:], NEG_LN128)

            for c in range(N_LOCAL // CHUNK_PTS):
                xat = xap.tile([N_ROWS, CHUNK_PTS], bf16)
                if c == 0:
                    # split so the first block's operands land ASAP
                    nc.gpsimd.dma_start(
                        out=xat[:, 0:BLOCK_PTS], in_=xa[:, 0:BLOCK_PTS]
                    )
                    nc.gpsimd.dma_start(
                        out=xat[:, BLOCK_PTS:CHUNK_PTS],
                        in_=xa[:, BLOCK_PTS:CHUNK_PTS],
                    )
                else:
                    nc.gpsimd.dma_start(
                        out=xat[:], in_=xa[:, c * CHUNK_PTS : (c + 1) * CHUNK_PTS]
                    )
                def emit_tree(vals_t, nq, g0):
                    # 16->1 grouped sum: 4-level pairwise add tree over the
                    # l-major layout (quarter q = 128 points, j = l*64+m),
                    # then one strided DMA writing all nq*128 output rows.
                    vq = vals_t[:, : nq * MK].rearrange("p (q x) -> p q x", q=nq)
                    t1 = t1p.tile([128, nq * 512], bf16)
                    t13 = t1[:].rearrange("p (q x) -> p q x", q=nq)
                    nc.vector.tensor_tensor(
                        out=t13, in0=vq[:, :, 0:512], in1=vq[:, :, 512:1024], op=ADD
                    )
                    t2 = t2p.tile([128, nq * 256], bf16)
                    t23 = t2[:].rearrange("p (q x) -> p q x", q=nq)
                    nc.vector.tensor_tensor(
                        out=t23, in0=t13[:, :, 0:256], in1=t13[:, :, 256:512], op=ADD
                    )
                    t3 = t3p.tile([128, nq * 128], bf16)
                    t33 = t3[:].rearrange("p (q x) -> p q x", q=nq)
                    nc.vector.tensor_tensor(
                        out=t33, in0=t23[:, :, 0:128], in1=t23[:, :, 128:256], op=ADD
                    )
                    ot = outp.tile([128, nq * M_KERN], f32)
                    ot3 = ot[:].rearrange("p (q x) -> p q x", q=nq)
                    nc.vector.tensor_tensor(
                        out=ot3, in0=t33[:, :, 0:64], in1=t33[:, :, 64:128], op=ADD
                    )
                    nc.sync.dma_start(
                        out=out[g0 : g0 + nq * 128, :].rearrange(
                            "(q p) m -> p q m", q=nq
                        ),
                        in_=ot3,
                    )

                last_chunk = c == N_LOCAL // CHUNK_PTS - 1
                for b in range(CHUNK_PTS // BLOCK_PTS):
                    g0 = c * CHUNK_PTS + b * BLOCK_PTS  # global row offset
                    last_block = last_chunk and b == CHUNK_PTS // BLOCK_PTS - 1
                    vals = valsp.tile([128, 4 * MK], bf16)
                    for s in range(2):  # two 256-pt PSUM sub-blocks
                        ps = psump.tile([128, 2 * MK], f32)
                        for half in range(2):
                            p0 = b * BLOCK_PTS + s * 256 + half * 128
                            lhsT = xat[:, p0 : p0 + 128]
                            for jb in range(2):
                                nc.tensor.matmul(
                                    out=ps[
                                        :,
                                        half * MK + jb * 512 : half * MK
                                        + (jb + 1) * 512,
                                    ],
                                    lhsT=lhsT,
                                    rhs=kat[:, jb * 512 : (jb + 1) * 512],
                                    start=True,
                                    stop=True,
                                )
                        if c == 0 and b == 0 and s == 0:
                            # two half-size ACTs so the exp stream starts as
                            # soon as the first two matmuls land
                            for h in range(2):
                                nc.scalar.activation(
                                    out=vals[:, h * MK : (h + 1) * MK],
                                    in_=ps[:, h * MK : (h + 1) * MK],
                                    func=EXP,
                                    bias=bias_t[:],
                                )
                        else:
                            nc.scalar.activation(
                                out=vals[:, s * 2 * MK : (s + 1) * 2 * MK],
                                in_=ps[:],
                                func=EXP,
                                bias=bias_t[:],
                            )
                        if last_block:
                            # per-sub-block trees shorten the post-ACT tail
                            emit_tree(vals[:, s * 2 * MK :], 2, g0 + s * 256)
                    if not last_block:
                        emit_tree(vals[:], 4, g0)
    return nc


def _split_bf16(a32):
    """fp32 array -> (hi, lo) bf16 pair with hi + lo ~= a32."""
    import ml_dtypes

    hi = a32.astype(ml_dtypes.bfloat16)
    lo = (a32 - hi.astype(np.float32)).astype(ml_dtypes.bfloat16)
    return hi, lo


def _prep_operands(normal, kern):
    """Build the rank-13 augmented bf16 operands so that
    (xa.T @ ka)[i, j] ~= 2 x_i.k_j - |x_i|^2 - |k_j|^2 = -d2[i, j].

    ka columns are l-major: j = l*64 + m, so the 16 summands of output
    group m sit at stride 64 -- each tree level adds two dense slabs."""
    import ml_dtypes

    x = np.ascontiguousarray(np.asarray(normal, dtype=np.float32))  # (n, 3)
    kf = np.asarray(kern, dtype=np.float32)  # (64, 16, 3)
    kf = np.ascontiguousarray(kf.transpose(1, 0, 2).reshape(MK, 3))  # l-major

    n2 = (x * x).sum(axis=1)  # (n,)
    k2 = (kf * kf).sum(axis=1)  # (1024,)

    xhi, xlo = _split_bf16(x)
    khi, klo = _split_bf16(kf)
    n2hi, n2lo = _split_bf16(n2)
    k2hi, k2lo = _split_bf16(k2)

    n = x.shape[0]
    ones_n = np.ones(n, dtype=ml_dtypes.bfloat16)
    ones_k = np.ones(MK, dtype=ml_dtypes.bfloat16)
    two_khi = (2.0 * khi.astype(np.float32)).astype(ml_dtypes.bfloat16)  # exact
    two_klo = (2.0 * klo.astype(np.float32)).astype(ml_dtypes.bfloat16)  # exact

    # row r of xa pairs with row r of ka; sum over the 13 rows gives -d2.
    xa = np.empty((N_ROWS, n), dtype=ml_dtypes.bfloat16)
    ka = np.empty((N_ROWS, MK), dtype=ml_dtypes.bfloat16)
    xa[0:3] = xhi.T
    ka[0:3] = two_khi.T
    xa[3:6] = xhi.T
    ka[3:6] = two_klo.T
    xa[6:9] = xlo.T
    ka[6:9] = two_khi.T
    xa[9] = -n2hi
    ka[9] = ones_k
    xa[10] = -n2lo
    ka[10] = ones_k
    xa[11] = ones_n
    ka[11] = -k2hi
    xa[12] = ones_n
    ka[12] = -k2lo
    return xa, ka


def kernel(normal, neighbour, kernel):  # noqa: A002 - harness-fixed names
    global _CACHED_NC, LAST_RESULTS
    from concourse.bass_utils import run_bass_kernel_spmd

    xa, ka = _prep_operands(normal, kernel)
    assert xa.shape[1] == N_TOTAL, xa.shape

    if _CACHED_NC is None:
        _CACHED_NC = _build_bass()
        if not _CACHED_NC.is_finalized():
            _CACHED_NC.finalize()

    in_maps = [
        {
            "xa": np.ascontiguousarray(xa[:, i * N_LOCAL : (i + 1) * N_LOCAL]),
            "ka": ka,
        }
        for i in range(N_CORES)
    ]
    res = run_bass_kernel_spmd(
        _CACHED_NC, in_maps, list(range(N_CORES)), trace=TRACE
    )
    LAST_RESULTS = res
    out = np.concatenate(
        [res.results[i]["out"] for i in range(N_CORES)], axis=0
    )
    return np.ascontiguousarray(out.astype(np.float32))


# revision 11
# speedup vs baseline: 1.0061x; 1.0061x over previous
"""Kernel-correlation (gnn_message_passing) Trainium2 kernel.

out[i, m] = (1/128) * sum_{l<16} exp(-||normal[i] - kernel[m, l]||^2)

Strategy (data-parallel over points, 8 NeuronCores, no collectives):
  -d2[i, j] = 2 x_i . k_j - |x_i|^2 - |k_j|^2 is a rank-13 product of two
  bf16 hi/lo-split augmented matrices (fp32-grade accuracy at full bf16 PE
  rate), computed straight into PSUM by the TensorEngine.  ScalarE then does
  exp(psum - ln 128) PSUM->SBUF in one pass (the 1/128 output scale rides the
  free activation bias), writing bf16.  The grouped 16->1 reduction is a
  4-level pairwise tensor_tensor add tree on VectorE: the kernel columns are
  laid out l-major (j = l*64 + m), so each level sums two dense contiguous
  bf16 slabs at the DVE 2x perf mode -- ~1.7x faster than the 1x-only
  tensor_reduce.  The last level emits fp32 and output rows DMA back
  contiguously.

Per core: 32768 points x 1024 kernel-points = 33.5M exp evals; ACT floor
~(N+352)/1.2GHz -> ~252us/core expected; DVE tree ~177us, PE ~110-220us.
"""

import math

import numpy as np

N_TOTAL = 262144
N_CORES = 8
N_LOCAL = N_TOTAL // N_CORES  # 32768
M_KERN = 64
K_SUB = 16
MK = M_KERN * K_SUB  # 1024
N_ROWS = 13  # 9 hi/lo cross terms + n2 hi/lo + k2 hi/lo
CHUNK_PTS = 2048  # points per input DMA chunk
BLOCK_PTS = 512  # points per reduction-tree block (4 quarters of 128)

TRACE = False  # set by test.py to collect a neuron profile
LAST_RESULTS = None  # BassKernelResults of the most recent run

_CACHED_NC = None


def _build_bass():
    import concourse.bacc as bacc
    import concourse.mybir as mybir
    from concourse.tile import TileContext

    f32 = mybir.dt.float32
    bf16 = mybir.dt.bfloat16
    EXP = mybir.ActivationFunctionType.Exp
    ADD = mybir.AluOpType.add

    nc = bacc.Bacc()
    xa = nc.declare_dram_parameter("xa", [N_ROWS, N_LOCAL], bf16, isOutput=False)
    ka = nc.declare_dram_parameter("ka", [N_ROWS, MK], bf16, isOutput=False)
    out = nc.declare_dram_parameter("out", [N_LOCAL, M_KERN], f32, isOutput=True)

    with TileContext(nc) as tc:
        with (
            tc.tile_pool(name="kap", bufs=1) as kap,
            tc.tile_pool(name="xap", bufs=2) as xap,
            tc.tile_pool(name="valsp", bufs=2) as valsp,
            tc.tile_pool(name="t1p", bufs=2) as t1p,
            tc.tile_pool(name="t2p", bufs=2) as t2p,
            tc.tile_pool(name="t3p", bufs=2) as t3p,
            tc.tile_pool(name="outp", bufs=3) as outp,
            tc.tile_pool(name="psump", bufs=2, space="PSUM") as psump,
        ):
            kat = kap.tile([N_ROWS, MK], bf16)
            nc.sync.dma_start(out=kat[:], in_=ka[:])

            for c in range(N_LOCAL // CHUNK_PTS):
                xat = xap.tile([N_ROWS, CHUNK_PTS], bf16)
                if c == 0:
                    # split so the first block's operands land ASAP
                    nc.gpsimd.dma_start(
                        out=xat[:, 0:BLOCK_PTS], in_=xa[:, 0:BLOCK_PTS]
                    )
                    nc.gpsimd.dma_start(
                        out=xat[:, BLOCK_PTS:CHUNK_PTS],
                        in_=xa[:, BLOCK_PTS:CHUNK_PTS],
                    )
                else:
                    nc.gpsimd.dma_start(
                        out=xat[:], in_=xa[:, c * CHUNK_PTS : (c + 1) * CHUNK_PTS]
                    )
                def emit_tree(vals_t, nq, g0):
                    # 16->1 grouped sum: 4-level pairwise add tree over the
                    # l-major layout (quarter q = 128 points, j = l*64+m),
                    # then one strided DMA writing all nq*128 output rows.
                    vq = vals_t[:, : nq * MK].rearrange("p (q x) -> p q x", q=nq)
                    t1 = t1p.tile([128, nq * 512], bf16)
                    t13 = t1[:].rearrange("p (q x) -> p q x", q=nq)
                    nc.vector.tensor_tensor(
                        out=t13, in0=vq[:, :, 0:512], in1=vq[:, :, 512:1024], op=ADD
                    )
                    t2 = t2p.tile([128, nq * 256], bf16)
                    t23 = t2[:].rearrange("p (q x) -> p q x", q=nq)
                    nc.vector.tensor_tensor(
                        out=t23, in0=t13[:, :, 0:256], in1=t13[:, :, 256:512], op=ADD
                    )
                    t3 = t3p.tile([128, nq * 128], bf16)
                    t33 = t3[:].rearrange("p (q x) -> p q x", q=nq)
                    nc.vector.tensor_tensor(
                        out=t33, in0=t23[:, :, 0:128], in1=t23[:, :, 128:256], op=ADD
                    )
                    ot = outp.tile([128, nq * M_KERN], f32)
                    ot3 = ot[:].rearrange("p (q x) -> p q x", q=nq)
                    nc.vector.tensor_tensor(
                        out=ot3, in0=t33[:, :, 0:64], in1=t33[:, :, 64:128], op=ADD
                    )
                    nc.sync.dma_start(
                        out=out[g0 : g0 + nq * 128, :].rearrange(
                            "(q p) m -> p q m", q=nq
                        ),
                        in_=ot3,
                    )

                last_chunk = c == N_LOCAL // CHUNK_PTS - 1
                for b in range(CHUNK_PTS // BLOCK_PTS):
                    g0 = c * CHUNK_PTS + b * BLOCK_PTS  # global row offset
                    last_block = last_chunk and b == CHUNK_PTS // BLOCK_PTS - 1
                    vals = valsp.tile([128, 4 * MK], bf16)
                    for s in range(2):  # two 256-pt PSUM sub-blocks
                        ps = psump.tile([128, 2 * MK], f32)
                        for half in range(2):
                            p0 = b * BLOCK_PTS + s * 256 + half * 128
                            lhsT = xat[:, p0 : p0 + 128]
                            for jb in range(2):
                                nc.tensor.matmul(
                                    out=ps[
                                        :,
                                        half * MK + jb * 512 : half * MK
                                        + (jb + 1) * 512,
                                    ],
                                    lhsT=lhsT,
                                    rhs=kat[:, jb * 512 : (jb + 1) * 512],
                                    start=True,
                                    stop=True,
                                )
                        nc.scalar.activation(
                            out=vals[:, s * 2 * MK : (s + 1) * 2 * MK],
                            in_=ps[:],
                            func=EXP,
                        )
                        if last_block:
                            # per-sub-block trees shorten the post-ACT tail
                            emit_tree(vals[:, s * 2 * MK :], 2, g0 + s * 256)
                    if not last_block:
                        emit_tree(vals[:], 4, g0)
    return nc


def _split_bf16(a32):
    """fp32 array -> (hi, lo) bf16 pair with hi + lo ~= a32."""
    import ml_dtypes

    hi = a32.astype(ml_dtypes.bfloat16)
    lo = (a32 - hi.astype(np.float32)).astype(ml_dtypes.bfloat16)
    return hi, lo


def _prep_operands(normal, kern):
    """Build the rank-13 augmented bf16 operands so that
    (xa.T @ ka)[i, j] ~= 2 x_i.k_j - |x_i|^2 - |k_j|^2 = -d2[i, j].

    ka columns are l-major: j = l*64 + m, so the 16 summands of output
    group m sit at stride 64 -- each tree level adds two dense slabs."""
    import ml_dtypes

    x = np.ascontiguousarray(np.asarray(normal, dtype=np.float32))  # (n, 3)
    kf = np.asarray(kern, dtype=np.float32)  # (64, 16, 3)
    kf = np.ascontiguousarray(kf.transpose(1, 0, 2).reshape(MK, 3))  # l-major

    n2 = (x * x).sum(axis=1)  # (n,)
    # fold the 1/128 output scale into the exponent: exp(-d2 - ln128)
    k2 = (kf * kf).sum(axis=1) + np.float32(math.log(128.0))  # (1024,)

    xhi, xlo = _split_bf16(x)
    khi, klo = _split_bf16(kf)
    n2hi, n2lo = _split_bf16(n2)
    k2hi, k2lo = _split_bf16(k2)

    n = x.shape[0]
    ones_n = np.ones(n, dtype=ml_dtypes.bfloat16)
    ones_k = np.ones(MK, dtype=ml_dtypes.bfloat16)
    two_khi = (2.0 * khi.astype(np.float32)).astype(ml_dtypes.bfloat16)  # exact
    two_klo = (2.0 * klo.astype(np.float32)).astype(ml_dtypes.bfloat16)  # exact

    # row r of xa pairs with row r of ka; sum over the 13 rows gives -d2.
    xa = np.empty((N_ROWS, n), dtype=ml_dtypes.bfloat16)
    ka = np.empty((N_ROWS, MK), dtype=ml_dtypes.bfloat16)
    xa[0:3] = xhi.T
    ka[0:3] = two_khi.T
    xa[3:6] = xhi.T
    ka[3:6] = two_klo.T
    xa[6:9] = xlo.T
    ka[6:9] = two_khi.T
    xa[9] = -n2hi
    ka[9] = ones_k
    xa[10] = -n2lo
    ka[10] = ones_k
    xa[11] = ones_n
    ka[11] = -k2hi
    xa[12] = ones_n
    ka[12] = -k2lo
    return xa, ka


def kernel(normal, neighbour, kernel):  # noqa: A002 - harness-fixed names
    global _CACHED_NC, LAST_RESULTS
    from concourse.bass_utils import run_bass_kernel_spmd

    xa, ka = _prep_operands(normal, kernel)
    assert xa.shape[1] == N_TOTAL, xa.shape

    if _CACHED_NC is None:
        _CACHED_NC = _build_bass()
        if not _CACHED_NC.is_finalized():
            _CACHED_NC.finalize()

    in_maps = [
        {
            "xa": np.ascontiguousarray(xa[:, i * N_LOCAL : (i + 1) * N_LOCAL]),
            "ka": ka,
        }
        for i in range(N_CORES)
    ]
    res = run_bass_kernel_spmd(
        _CACHED_NC, in_maps, list(range(N_CORES)), trace=TRACE
    )
    LAST_RESULTS = res
    out = np.concatenate(
        [res.results[i]["out"] for i in range(N_CORES)], axis=0
    )
    return np.ascontiguousarray(out.astype(np.float32))
